# revision 1
# baseline (speedup 1.0000x reference)
"""Trainium2 Bass kernel for nn_DQN CEM sampling problem (v3).

Data-parallel over batch: 4096 rows -> 8 cores x 512 rows. Each core runs the
full 99-step CEM loop on its shard; the tiny MLP weights are replicated.

This hardware's PE activity throttler caps sustained matmul streams at
K=4/8 (1.2 GHz effective), so the optimization goal is fewer PE cycles and
cheaper PSUM drains, not HAM warm-up:
  - fp16 matmuls; angle fed as hi+lo fp16 pair for ~fp32 precision; biases
    folded into the weights via a constant-1 input row (homogeneous coords).
  - l1 (K=5) runs as 4x ROW-TILED concurrent quads: tile_position (32r, 0)
    with x and W1 replicated on partition stripes 0/32/64/96. 4 matmuls run
    concurrently in the 32x128 array mode.
  - l3 (M=1, padded 32) runs as 4x COL-TILED concurrent quads: 400-wide
    tiles (8 rows x 50 samples) at tile_position (0, 32s) for the 4 streams
    (group x tile-parity); q lands on psum partitions 0/32/64/96 and flips
    to batch-major with 4 strided DMAs.
  - PSUM->SBUF relu drains on DVE use tensor_tensor max-with-zero (single
    ALU pass) instead of tensor_scalar (two passes); drains split DVE/ACT
    by greedy cost balance with measured per-op costs.
  - bitonic top-32 + sampling arithmetic on the otherwise idle GpSimd.
  - For_i body unrolled BODY_U steps to cut all-engine barrier frequency.
"""

import numpy as np

BATCH = 4096
M = 50
NTOP = 32
ITERS = 100  # reference ITERS; device runs ITERS-1 = 99 qnet/stats steps
HIDDEN = 100
NCORES = 8
B = BATCH // NCORES  # 512 rows per core
G = 4                # partition groups per core
P = 128              # rows per group (partitions)
NPG = P * M          # columns per group = 6400
N = G * NPG          # columns per core = 25600
NPAIR = 2 * NPG      # columns per pair = 12800
NEG = -1.0e30
TWO_PI = 6.283185307179586

BODY_U = 3           # CEM steps per For_i body
GPSIMD_TAIL = False  # gpsimd rejects TensorTensor (walrus engine check)
TT_DRAIN = False     # measured: TT-max == tensor_scalar == ACT (~1.2ns/elem)
L1_QUAD = True       # l1 as 4x row-tiled concurrent quads
L3_QUAD = True       # l3 as 4x col-tiled concurrent quads (400-wide)

_PROG_CACHE = {}


def _tiles(total, width):
    out = []
    off = 0
    while off < total:
        w = min(width, total - off)
        out.append((off, w))
        off += w
    return out


def build_program(n_steps=ITERS - 1):
    """Build the single-core Bass/Tile program (SPMD across cores)."""
    import concourse.bacc as bacc
    import concourse.bass as bass
    import concourse.tile as tile
    import concourse.mybir as mybir

    f32 = mybir.dt.float32
    fp16 = mybir.dt.float16
    Alu = mybir.AluOpType
    Act = mybir.ActivationFunctionType

    nc = bacc.Bacc("TRN2", target_bir_lowering=False, debug=False)

    XROWS = 101 if L1_QUAD else 5
    XA = nc.dram_tensor("XA", [XROWS, N], fp16, kind="ExternalInput")
    EPS = nc.dram_tensor("EPS", [max(n_steps - 1, 1), 2, P, 2 * M], f32,
                         kind="ExternalInput")
    W1D = nc.dram_tensor("W1D", [XROWS, HIDDEN], fp16, kind="ExternalInput")
    W2D = nc.dram_tensor("W2D", [HIDDEN + 1, HIDDEN], fp16,
                         kind="ExternalInput")
    W3C = 32 if L3_QUAD else 64
    W3D = nc.dram_tensor("W3D", [HIDDEN + 1, W3C], fp16, kind="ExternalInput")
    OUT = nc.dram_tensor("OUT", [B], f32, kind="ExternalOutput")

    sts = _tiles(NPAIR, 1024)      # supertiles for l1/l2 (12x1024 + 512)

    # measured per-op engine costs (ns) for the greedy drain balance
    def dve_cost(w, tt):
        return 175 + 1.20 * w

    def act_cost(w):
        return 180 + 1.13 * w

    with tile.TileContext(nc) as tc:
        with (
            tc.tile_pool(name="statics", bufs=1) as statics,
            tc.tile_pool(name="hps", bufs=3, space=bass.MemorySpace.PSUM) as hps,
            tc.tile_pool(name="psq", bufs=2, space=bass.MemorySpace.PSUM) as psq,
        ):
            # --- static tiles ---
            x = statics.tile([XROWS, N], fp16)    # s0, s1, a_hi, a_lo, 1 (striped)
            h1 = statics.tile([HIDDEN + 1, NPAIR], fp16)
            h2 = statics.tile([HIDDEN + 1, NPAIR], fp16)
            q_sbA = statics.tile([P, NPG], f32)
            q_sbB = statics.tile([P, NPG], f32)
            q64 = statics.tile([P, G, 64], f32)   # batch-major q + pad
            SA = statics.tile([P, G * 64], f32)
            SB = statics.tile([P, G * 64], f32)
            top32 = statics.tile([P, G, NTOP], f32)
            bnst = statics.tile([P, G, 6], f32)
            mv = statics.tile([P, G, 2], f32)     # (mean, var) per group
            std = statics.tile([P, G], f32)
            a_bm = statics.tile([P, G, M], f32)
            tmp_s = statics.tile([P, G, M], f32)
            a16 = statics.tile([P, G, 2, M], fp16)  # (hi, lo)
            eps_sbA = statics.tile([P, 2 * M], f32)
            eps_sbB = statics.tile([P, 2 * M], f32)
            out_sb = statics.tile([P, G], f32)
            zcol = statics.tile([P, 1], f32)
            w1s = statics.tile([XROWS, HIDDEN], fp16)
            w2s = statics.tile([HIDDEN + 1, HIDDEN], fp16)
            w3s = statics.tile([HIDDEN + 1, W3C], fp16)

            q_sbs = (q_sbA, q_sbB)
            eps_sbs = (eps_sbA, eps_sbB)

            ENG = nc.gpsimd if GPSIMD_TAIL else nc.vector

            # --- one-time setup ---
            nc.sync.dma_start(out=w1s, in_=W1D.ap())
            nc.sync.dma_start(out=w2s, in_=W2D.ap())
            nc.sync.dma_start(out=w3s, in_=W3D.ap())
            nc.sync.dma_start(out=x, in_=XA.ap())
            nc.vector.memset(q64[:, :, M:64], NEG)
            nc.vector.memset(zcol, 0.0)
            # rows 96..99 are clobbered but rewritten by the first l1/l2
            # drain before any consumer reads them; row 100 stays 1.0
            # (engine APs need a 32-aligned base partition).
            nc.vector.memset(h1[96:HIDDEN + 1, :], 1.0)
            nc.vector.memset(h2[96:HIDDEN + 1, :], 1.0)

            def load_eps(pair, t):
                if isinstance(t, int):
                    src = EPS.ap()[t:t + 1, pair:pair + 1, :, :]
                else:
                    src = EPS.ap()[bass.ds(t, 1), pair:pair + 1, :, :]
                # gpsimd queue: keeps bulky flips off the sync queue's FIFO
                nc.gpsimd.dma_start(out=eps_sbs[pair], in_=src)

            bal = {"dve": 0.0, "act": 0.0}

            def drain(kind, out_ap, in_ap, w):
                """Emit a relu/copy drain on the engine with less queued work."""
                dc = dve_cost(w, TT_DRAIN)
                ac = act_cost(w)
                if bal["dve"] + dc <= bal["act"] + ac:
                    bal["dve"] += dc
                    if kind == "relu" and TT_DRAIN:
                        zb = zcol[0:in_ap.shape[0], :].to_broadcast(
                            (in_ap.shape[0], w))
                        nc.vector.tensor_tensor(out_ap, in_ap, zb, op=Alu.max)
                    elif kind == "relu":
                        nc.vector.tensor_scalar(out_ap, in_ap, scalar1=0.0,
                                                scalar2=None, op0=Alu.max)
                    else:
                        nc.vector.tensor_copy(out_ap, in_ap)
                else:
                    bal["act"] += ac
                    if kind == "relu":
                        nc.scalar.activation(out_ap, in_ap, Act.Relu)
                    else:
                        nc.scalar.copy(out_ap, in_ap)

            def mlp_units(pair):
                """List of emission units (closures) for this pair's MLP."""
                base = pair * NPAIR
                qsb = q_sbs[pair]
                units = []

                # ---- l1 block ----
                if L1_QUAD:
                    # 4x row-tiled: quad of concurrent matmuls covers two
                    # 1024-supertiles; stripe r reads x/W1 at partitions 32r.
                    i = 0
                    while i < len(sts):
                        pair_sts = sts[i:i + 2]

                        def l1_quad(pair_sts=pair_sts):
                            tiles_ = []
                            for _ in pair_sts:
                                hst = hps.tile([HIDDEN, 1024], f32,
                                               tag="hst", name="hst")
                                tiles_.append(hst)
                            r = 0
                            for ti, (off, w) in enumerate(pair_sts):
                                for w0 in range(0, w, 512):
                                    ww = min(512, w - w0)
                                    c0 = base + off + w0
                                    nc.tensor.matmul(
                                        tiles_[ti][:, w0:w0 + ww],
                                        w1s[32 * r:32 * r + 5, :],
                                        x[32 * r:32 * r + 5, c0:c0 + ww],
                                        tile_position=(32 * r, 0))
                                    r += 1
                            for ti, (off, w) in enumerate(pair_sts):
                                drain("relu", h1[0:HIDDEN, off:off + w],
                                      tiles_[ti][:, 0:w], w)
                        units.append(l1_quad)
                        i += 2
                else:
                    for (off, w) in sts:
                        def l1_unit(off=off, w=w):
                            st = hps.tile([HIDDEN, 1024], f32, tag="hst")
                            for w0 in range(0, w, 512):
                                ww = min(512, w - w0)
                                c0 = base + off + w0
                                nc.tensor.matmul(st[:, w0:w0 + ww],
                                                 w1s[0:5, :], x[0:5, c0:c0 + ww])
                            drain("relu", h1[0:HIDDEN, off:off + w],
                                  st[:, 0:w], w)
                        units.append(l1_unit)

                # ---- l2 block ----
                for (off, w) in sts:
                    def l2_unit(off=off, w=w):
                        st = hps.tile([HIDDEN, 1024], f32, tag="hst")
                        for w0 in range(0, w, 512):
                            ww = min(512, w - w0)
                            nc.tensor.matmul(st[:, w0:w0 + ww], w2s,
                                             h1[:, off + w0:off + w0 + ww])
                        drain("relu", h2[0:HIDDEN, off:off + w], st[:, 0:w], w)
                    units.append(l2_unit)

                # ---- l3 block ----
                if L3_QUAD:
                    # 4x col-tiled: strip s = (group j, row-half h); tile
                    # tau = 8h+T covers rows [8*tau, 8*tau+8) of group j, so
                    # each strip's q is a contiguous 64-partition run after
                    # the flip (nested partition APs don't lower for DMA).
                    for T in range(8):
                        def l3_quad(T=T):
                            qp = psq.tile([P, 400], f32, tag="qp")
                            for s in range(4):
                                j, h = s // 2, s % 2
                                tau = 8 * h + T
                                c0 = j * NPG + tau * 400
                                nc.tensor.matmul(
                                    qp[32 * s:32 * s + 32, :], w3s,
                                    h2[:, c0:c0 + 400],
                                    tile_position=(0, 32 * s))
                            drain("copy", qsb[:, 400 * T:400 * T + 400],
                                  qp[:, :], 400)
                        units.append(l3_quad)
                else:
                    for (off, w) in _tiles(NPG, 512):
                        def l3_unit(off=off, w=w):
                            qp = psq.tile([P, 512], f32, tag="qp")
                            for j in range(2):
                                nc.tensor.matmul(
                                    qp[64 * j:64 * j + 64, :w], w3s,
                                    h2[:, j * NPG + off:j * NPG + off + w],
                                    tile_position=(0, 64 * j))
                            drain("copy", qsb[:, off:off + w], qp[:, :w], w)
                        units.append(l3_unit)
                return units

            def tail_thunks(pair, do_sample):
                """Top-k/stats/sample for this pair's q (already in q_sb)."""
                g0 = 2 * pair
                qsb = q_sbs[pair]
                ths = []

                if L3_QUAD:
                    for s in range(4):
                        j, h = s // 2, s % 2

                        def qflip(s=s, j=j, h=h):
                            nc.gpsimd.dma_start(
                                out=q64[64 * h:64 * h + 64, g0 + j, 0:M],
                                in_=qsb[32 * s:32 * s + 1, 0:3200])
                        ths.append(qflip)
                else:
                    for j in range(2):
                        def qflip(j=j):
                            nc.gpsimd.dma_start(
                                out=q64[:, g0 + j, 0:M],
                                in_=qsb[64 * j:64 * j + 1, :].rearrange(
                                    "a (p m) -> a p m", m=M))
                        ths.append(qflip)

                def hv(t):
                    return t[:, pair * 128:(pair + 1) * 128].rearrange(
                        "p (h m) -> p h m", m=32)

                src = hv(q64.rearrange("p g m -> p (g m)"))
                dst_list = [hv(SA), hv(SB)]
                which = 0
                for k in [2, 4, 8, 16, 32]:
                    dst = dst_list[which]; which ^= 1
                    s4 = src.rearrange("p h (nb k) -> p h nb k", k=k)
                    d4 = dst.rearrange("p h (nb k) -> p h nb k", k=k)

                    def flip_max(d4=d4, s4=s4, k=k):
                        nc.vector.tensor_tensor(
                            d4[:, :, :, 0:k // 2], s4[:, :, :, 0:k // 2],
                            s4[:, :, :, k - 1:k // 2 - 1:-1], op=Alu.max)

                    def flip_min(d4=d4, s4=s4, k=k):
                        nc.vector.tensor_tensor(
                            d4[:, :, :, k // 2:k], s4[:, :, :, k // 2:k],
                            s4[:, :, :, k // 2 - 1::-1], op=Alu.min)
                    ths += [flip_max, flip_min]
                    src = dst
                    d = k // 4
                    while d >= 1:
                        dst = dst_list[which]; which ^= 1
                        s5 = src.rearrange("p h (nb two d) -> p h nb two d",
                                           two=2, d=d)
                        d5 = dst.rearrange("p h (nb two d) -> p h nb two d",
                                           two=2, d=d)

                        def plain_max(d5=d5, s5=s5):
                            ENG.tensor_tensor(
                                d5[:, :, :, 0, :], s5[:, :, :, 0, :],
                                s5[:, :, :, 1, :], op=Alu.max)

                        def plain_min(d5=d5, s5=s5):
                            ENG.tensor_tensor(
                                d5[:, :, :, 1, :], s5[:, :, :, 0, :],
                                s5[:, :, :, 1, :], op=Alu.min)
                        ths += [plain_max, plain_min]
                        src = dst
                        d //= 2

                sg = src.rearrange("p (g h) m -> p g h m", g=2)

                def merge(sg=sg):
                    nc.vector.tensor_tensor(top32[:, g0:g0 + 2, :],
                                      sg[:, :, 0, :],
                                      sg[:, :, 1, ::-1], op=Alu.max)
                ths.append(merge)

                for g in (g0, g0 + 1):
                    def bns(g=g):
                        nc.vector.bn_stats(bnst[:, g, :], top32[:, g, :])

                    def bna(g=g):
                        nc.vector.bn_aggr(mv[:, g, :], bnst[:, g:g + 1, :])
                    ths += [bns, bna]

                if do_sample:
                    def sqrt_op():
                        nc.scalar.activation(std[:, g0:g0 + 2],
                                             mv[:, g0:g0 + 2, 1], Act.Sqrt,
                                             scale=float(NTOP) / (NTOP - 1))
                    ths.append(sqrt_op)

                    epsv = eps_sbs[pair].rearrange("p (g m) -> p g m", m=M)
                    stdb = std[:, g0:g0 + 2].unsqueeze(2).to_broadcast(
                        (P, 2, M))
                    mub = mv[:, g0:g0 + 2, 0].unsqueeze(2).to_broadcast(
                        (P, 2, M))

                    def smul(epsv=epsv, stdb=stdb):
                        ENG.tensor_tensor(tmp_s[:, g0:g0 + 2, :], epsv,
                                          stdb, op=Alu.mult)
                    ths.append(smul)

                    def sadd(mub=mub):
                        ENG.tensor_tensor(a_bm[:, g0:g0 + 2, :],
                                          tmp_s[:, g0:g0 + 2, :], mub,
                                          op=Alu.add)
                    ths.append(sadd)

                    def hi_cast():
                        nc.vector.tensor_scalar(a16[:, g0:g0 + 2, 0, :],
                                                a_bm[:, g0:g0 + 2, :],
                                                scalar1=0.0, scalar2=None,
                                                op0=Alu.add)
                    ths.append(hi_cast)

                    def lo_sub():
                        nc.vector.tensor_tensor(a16[:, g0:g0 + 2, 1, :],
                                                a_bm[:, g0:g0 + 2, :],
                                                a16[:, g0:g0 + 2, 0, :],
                                                op=Alu.subtract)
                    ths.append(lo_sub)

                    for j in range(2):
                        for r in range(2):
                            def aflip(j=j, r=r):
                                g = g0 + j
                                nc.sync.dma_start(
                                    out=x[2 + r:3 + r,
                                          g * NPG:(g + 1) * NPG].rearrange(
                                        "a (p m) -> a p m", m=M),
                                    in_=a16[:, g, r, :])
                            ths.append(aflip)

                    if L1_QUAD:
                        # replicate the fresh angle rows to partition
                        # stripes 32/64/96 for the row-tiled l1
                        cols = slice(g0 * NPG, (g0 + 2) * NPG)
                        for r in (1, 2, 3):
                            def stripe(r=r, cols=cols):
                                nc.gpsimd.dma_start(
                                    out=x[32 * r + 2:32 * r + 4, cols],
                                    in_=x[2:4, cols])
                            ths.append(stripe)
                return ths

            def phase(mlp_pair, tail):
                """Emit one pair's MLP with the other pair's tail interleaved."""
                bal["dve"] = 0.0
                bal["act"] = 0.0
                thunks = tail_thunks(*tail) if tail is not None else []
                if mlp_pair is None:
                    for th in thunks:
                        th()
                    return
                units = mlp_units(mlp_pair)
                nt = len(units)
                per = 2 if thunks else 0
                ti = 0
                for u in units:
                    u()
                    ti += 1
                    lo = per * (ti - 1)
                    for th in thunks[lo:lo + per]:
                        th()
                for th in thunks[per * ti:]:
                    th()

            # ---- prologue: t = 0 ----
            phase(0, None)
            if n_steps > 1:
                load_eps(0, 0)
            phase(1, (0, n_steps > 1))

            # ---- t = 1 .. : unrolled pipelined loop ----
            n_loop = n_steps - 3
            n_bodies = max(n_loop // BODY_U, 0)
            loop_end = 1 + n_bodies * BODY_U
            if n_bodies > 0:
                with tc.For_i(1, loop_end, BODY_U,
                              hint_engines=(mybir.EngineType.PE,)) as it:
                    for c in range(BODY_U):
                        load_eps(1, it + (c - 1))
                        phase(0, (1, True))
                        load_eps(0, it + c)
                        phase(1, (0, True))

            for t in range(loop_end, n_steps - 1):
                load_eps(1, t - 1)
                phase(0, (1, True))
                load_eps(0, t)
                phase(1, (0, True))

            # ---- epilogue: t = n_steps-1 ----
            if n_steps > 1:
                load_eps(1, n_steps - 2)
                phase(0, (1, True))
                phase(1, (0, False))      # pair A final stats
            phase(None, (1, False))       # pair B final stats
            nc.vector.tensor_scalar(out_sb, mv[:, :, 0], scalar1=TWO_PI,
                                    scalar2=None, op0=Alu.mult)
            nc.sync.dma_start(out=OUT.ap().rearrange("(g p) -> p g", p=P),
                              in_=out_sb)

    nc.compile()
    return nc


def host_prng(n_steps=ITERS - 1):
    """Exactly the reference's PRNG stream, on host CPU."""
    import jax
    import jax.numpy as jnp
    cpu = jax.devices("cpu")[0]
    with jax.default_device(cpu):
        key = jax.device_put(jax.random.key(42), cpu)
        k0, kloop = jax.random.split(key)
        angles0 = np.asarray(jax.random.uniform(k0, (BATCH, M),
                                                dtype=jnp.float32))
        keys = jax.random.split(kloop, ITERS - 1)
        eps = np.stack([
            np.asarray(jax.random.normal(keys[t], (BATCH, M),
                                         dtype=jnp.float32))
            for t in range(max(n_steps - 1, 1))
        ])
    return angles0, eps


def make_in_map(core, states, W1, b1, W2, b2, W3, b3, angles0, eps):
    sl = slice(core * B, (core + 1) * B)
    S = np.ascontiguousarray(states[sl]).reshape(G, P, 2)
    xrep = np.ascontiguousarray(
        np.broadcast_to(S[:, :, None, :], (G, P, M, 2)).transpose(3, 0, 1, 2)
    ).reshape(2, N)
    a0 = np.ascontiguousarray(angles0[sl]).reshape(N).astype(np.float32)
    a0_hi = a0.astype(np.float16)
    a0_lo = (a0 - a0_hi.astype(np.float32)).astype(np.float16)
    stripe = np.concatenate([
        xrep.astype(np.float16),
        a0_hi[None, :],
        a0_lo[None, :],
        np.ones((1, N), np.float16),
    ], axis=0)                      # [5, N]
    w1stripe = np.stack([W1[0], W1[1], W1[2], W1[2], b1]).astype(np.float16)
    if L1_QUAD:
        xa = np.zeros((101, N), np.float16)
        w1p = np.zeros((101, HIDDEN), np.float16)
        for r in range(4):
            xa[32 * r:32 * r + 5] = stripe
            w1p[32 * r:32 * r + 5] = w1stripe
    else:
        xa = stripe
        w1p = w1stripe
    nsteps_eps = max(eps.shape[0], 1)
    epsc = np.ascontiguousarray(
        eps[:, sl, :].reshape(nsteps_eps, 2, 2, P, M)
        .transpose(0, 1, 3, 2, 4)
    ).reshape(nsteps_eps, 2, P, 2 * M)
    w2p = np.concatenate([W2, b2[None, :]], axis=0).astype(np.float16)
    w3c = 32 if L3_QUAD else 64
    w3p = np.zeros((HIDDEN + 1, w3c), np.float16)
    w3p[0:HIDDEN, 0] = W3[:, 0].astype(np.float16)
    w3p[HIDDEN, 0] = np.float16(b3[0])
    return {
        "XA": xa,
        "EPS": epsc.astype(np.float32),
        "W1D": w1p,
        "W2D": w2p,
        "W3D": w3p,
    }


LAST_RESULTS = None


def kernel(states, W1, b1, W2, b2, W3, b3, _trace=False):
    global LAST_RESULTS
    from concourse.bass_utils import run_bass_kernel_spmd

    n_steps = ITERS - 1
    if n_steps not in _PROG_CACHE:
        _PROG_CACHE[n_steps] = build_program(n_steps)
    nc = _PROG_CACHE[n_steps]

    angles0, eps = host_prng(n_steps)
    in_maps = [
        make_in_map(c, states, W1, b1, W2, b2, W3, b3, angles0, eps)
        for c in range(NCORES)
    ]
    res = run_bass_kernel_spmd(nc, in_maps, core_ids=list(range(NCORES)),
                               trace=_trace)
    LAST_RESULTS = res
    out = np.concatenate([res.results[c]["OUT"] for c in range(NCORES)])
    return out.astype(np.float32)



# revision 9
# speedup vs baseline: 1.0269x; 1.0269x over previous
"""Trainium2 Bass kernel for nn_DQN CEM sampling problem (v4).

Data-parallel over batch: 4096 rows -> 8 cores x 512 rows. Each core runs the
full 99-step CEM loop on its shard; the tiny MLP weights are replicated.

v4 changes over v3 (trace-driven):
  - The phase's PSUM->SBUF drains (l1+l2 relu, l3 copy; ~33us/phase over
    DVE+ACT) are the hard floor, so the l1/l2/l3 units are emitted
    INTERLEAVED (quad, l2, l2, quad, ...) to keep both drain engines fed
    continuously instead of the v3 see-saw (l1 drain-bound then l2 PE-bound).
  - top-32 via DVE max8 + match_replace (4+3 ops per group) instead of the
    26-op bitonic network: shorter serial tail, ~2x fewer DVE-cycles.
  - sampling arithmetic (eps*std+mu) moved to the otherwise idle GpSimd
    (TT mult/add run on the Q7 cores; max/STT are rejected by walrus).
  - drain balance is tail-aware: DVE's greedy-cost counter starts
    preloaded with the tail work it must also absorb that phase.
  - the tail chain (sort/stats/sample/angle-writeback) is emitted at the
    HEAD of the next phase, not spread through it: its inputs are ready
    (q was flipped by qflips at the end of the owning pair's MLP phase),
    so it runs concurrently with the early drains instead of queueing
    behind all of them and gating the following phase's l1 by ~20us.
  - BODY_U 3 -> 8: each For_i back-edge is an all-engine barrier; since
    angle writeback completes mid-phase, the barrier exposes only ~2us.
"""

import numpy as np

BATCH = 4096
M = 50
NTOP = 32
ITERS = 100  # reference ITERS; device runs ITERS-1 = 99 qnet/stats steps
HIDDEN = 100
NCORES = 8
B = BATCH // NCORES  # 512 rows per core
G = 4                # partition groups per core
P = 128              # rows per group (partitions)
NPG = P * M          # columns per group = 6400
N = G * NPG          # columns per core = 25600
NPAIR = 2 * NPG      # columns per pair = 12800
NEG = -1.0e30
TWO_PI = 6.283185307179586

BODY_U = 8           # CEM steps per For_i body (back-edge = all-engine barrier)

_PROG_CACHE = {}


def _tiles(total, width):
    out = []
    off = 0
    while off < total:
        w = min(width, total - off)
        out.append((off, w))
        off += w
    return out


def build_program(n_steps=ITERS - 1):
    """Build the single-core Bass/Tile program (SPMD across cores)."""
    import concourse.bacc as bacc
    import concourse.bass as bass
    import concourse.tile as tile
    import concourse.mybir as mybir

    f32 = mybir.dt.float32
    fp16 = mybir.dt.float16
    Alu = mybir.AluOpType
    Act = mybir.ActivationFunctionType

    nc = bacc.Bacc("TRN2", target_bir_lowering=False, debug=False)

    XROWS = 101
    XA = nc.dram_tensor("XA", [XROWS, N], fp16, kind="ExternalInput")
    EPS = nc.dram_tensor("EPS", [max(n_steps - 1, 1), 2, P, 2 * M], f32,
                         kind="ExternalInput")
    W1D = nc.dram_tensor("W1D", [XROWS, HIDDEN], fp16, kind="ExternalInput")
    W2D = nc.dram_tensor("W2D", [HIDDEN + 1, HIDDEN], fp16,
                         kind="ExternalInput")
    W3C = 32
    W3D = nc.dram_tensor("W3D", [HIDDEN + 1, W3C], fp16, kind="ExternalInput")
    OUT = nc.dram_tensor("OUT", [B], f32, kind="ExternalOutput")

    sts = _tiles(NPAIR, 1024)      # supertiles for l1/l2 (12x1024 + 512)
    NST = len(sts)                 # 13

    # measured per-op engine costs (ns) for the greedy drain balance
    def dve_cost(w):
        return 175 + 1.20 * w

    def act_cost(w):
        return 180 + 1.13 * w

    # tail work that lands on each engine besides drains (ns), used to
    # preload the balance counters so ACT takes a bigger drain share.
    TAIL_DVE_NS = 3500                           # sort + bn + hi/lo casts
    TAIL_ACT_NS = 300                            # sqrt

    with tile.TileContext(nc) as tc:
        with (
            tc.tile_pool(name="statics", bufs=1) as statics,
            tc.tile_pool(name="hps", bufs=3, space=bass.MemorySpace.PSUM) as hps,
            tc.tile_pool(name="psq", bufs=2, space=bass.MemorySpace.PSUM) as psq,
        ):
            # --- static tiles ---
            x = statics.tile([XROWS, N], fp16)    # s0, s1, a_hi, a_lo, 1 (striped)
            h1 = statics.tile([HIDDEN + 1, NPAIR], fp16)
            h2 = statics.tile([HIDDEN + 1, NPAIR], fp16)
            q_sbA = statics.tile([P, NPG], f32)
            q_sbB = statics.tile([P, NPG], f32)
            q64 = statics.tile([P, G, 64], f32)   # batch-major q (50 used)
            srt = statics.tile([P, G, 64], f32)   # match_replace ping-pong
            top32 = statics.tile([P, G * NTOP], f32)
            bnst = statics.tile([P, G, 6], f32)
            mv = statics.tile([P, G, 2], f32)     # (mean, var) per group
            std = statics.tile([P, G], f32)
            a_bm = statics.tile([P, G, M], f32)
            tmp_s = statics.tile([P, G, M], f32)
            a16 = statics.tile([P, G, 2, M], fp16)  # (hi, lo)
            eps_sbA = statics.tile([P, 2 * M], f32)
            eps_sbB = statics.tile([P, 2 * M], f32)
            out_sb = statics.tile([P, G], f32)
            w1s = statics.tile([XROWS, HIDDEN], fp16)
            w2s = statics.tile([HIDDEN + 1, HIDDEN], fp16)
            w3s = statics.tile([HIDDEN + 1, W3C], fp16)

            q_sbs = (q_sbA, q_sbB)
            eps_sbs = (eps_sbA, eps_sbB)

            # --- one-time setup ---
            nc.sync.dma_start(out=w1s, in_=W1D.ap())
            nc.sync.dma_start(out=w2s, in_=W2D.ap())
            nc.sync.dma_start(out=w3s, in_=W3D.ap())
            nc.sync.dma_start(out=x, in_=XA.ap())
            # rows 96..99 are clobbered but rewritten by the first l1/l2
            # drain before any consumer reads them; row 100 stays 1.0
            # (engine APs need a 32-aligned base partition).
            nc.vector.memset(h1[96:HIDDEN + 1, :], 1.0)
            nc.vector.memset(h2[96:HIDDEN + 1, :], 1.0)

            def load_eps(pair, t):
                if isinstance(t, int):
                    src = EPS.ap()[t:t + 1, pair:pair + 1, :, :]
                else:
                    src = EPS.ap()[bass.ds(t, 1), pair:pair + 1, :, :]
                # gpsimd queue: keeps bulky flips off the sync queue's FIFO
                nc.gpsimd.dma_start(out=eps_sbs[pair], in_=src)

            bal = {"dve": 0.0, "act": 0.0}

            def drain(kind, out_ap, in_ap, w):
                """Emit a relu/copy drain on the engine with less queued work."""
                dc = dve_cost(w)
                ac = act_cost(w)
                if bal["dve"] + dc <= bal["act"] + ac:
                    bal["dve"] += dc
                    if kind == "relu":
                        nc.vector.tensor_scalar(out_ap, in_ap, scalar1=0.0,
                                                scalar2=None, op0=Alu.max)
                    else:
                        nc.vector.tensor_copy(out_ap, in_ap)
                else:
                    bal["act"] += ac
                    if kind == "relu":
                        nc.scalar.activation(out_ap, in_ap, Act.Relu)
                    else:
                        nc.scalar.copy(out_ap, in_ap)

            def mlp_units(pair):
                """Interleaved emission units (closures) for this pair's MLP.

                Order: q0 q1 l2_0 q2 l2_1 l2_2 q3 l2_3 l2_4 q4 l2_5 l2_6
                       q5 l2_7 l2_8 q6 l2_9 l2_10 T0 T1 T2 T3 l2_11 T4 T5
                       l2_12 T6 T7 -- keeps PE ~1 supertile ahead of the
                       drains so DVE/ACT (the floor) never starve, and l3
                       tiles start as soon as their h2 columns land.
                """
                base = pair * NPAIR
                qsb = q_sbs[pair]

                def l1_quad(qi):
                    # 4x row-tiled: quad of concurrent matmuls covers two
                    # 1024-supertiles; stripe r reads x/W1 at partitions 32r.
                    pair_sts = sts[2 * qi:2 * qi + 2]

                    def emit():
                        tiles_ = []
                        for _ in pair_sts:
                            hst = hps.tile([HIDDEN, 1024], f32,
                                           tag="hst", name="hst")
                            tiles_.append(hst)
                        r = 0
                        for ti, (off, w) in enumerate(pair_sts):
                            for w0 in range(0, w, 512):
                                ww = min(512, w - w0)
                                c0 = base + off + w0
                                nc.tensor.matmul(
                                    tiles_[ti][:, w0:w0 + ww],
                                    w1s[32 * r:32 * r + 5, :],
                                    x[32 * r:32 * r + 5, c0:c0 + ww],
                                    tile_position=(32 * r, 0))
                                r += 1
                        for ti, (off, w) in enumerate(pair_sts):
                            drain("relu", h1[0:HIDDEN, off:off + w],
                                  tiles_[ti][:, 0:w], w)
                    return emit

                def l2_unit(k):
                    off, w = sts[k]

                    def emit():
                        st = hps.tile([HIDDEN, 1024], f32, tag="hst")
                        for w0 in range(0, w, 512):
                            ww = min(512, w - w0)
                            nc.tensor.matmul(st[:, w0:w0 + ww], w2s,
                                             h1[:, off + w0:off + w0 + ww])
                        drain("relu", h2[0:HIDDEN, off:off + w], st[:, 0:w], w)
                    return emit

                def l3_quad(T):
                    # 4x col-tiled: strip s = (group j, row-half h); tile
                    # tau = 8h+T covers rows [8*tau, 8*tau+8) of group j, so
                    # each strip's q is a contiguous 64-partition run after
                    # the flip (nested partition APs don't lower for DMA).
                    def emit():
                        qp = psq.tile([P, 400], f32, tag="qp")
                        for s in range(4):
                            j, h = s // 2, s % 2
                            tau = 8 * h + T
                            c0 = j * NPG + tau * 400
                            nc.tensor.matmul(
                                qp[32 * s:32 * s + 32, :], w3s,
                                h2[:, c0:c0 + 400],
                                tile_position=(0, 32 * s))
                        drain("copy", qsb[:, 400 * T:400 * T + 400],
                              qp[:, :], 400)
                    return emit

                def qflips():
                    # flip q to batch-major as soon as the last l3 copy
                    # lands, so the next phase's tail starts with max8
                    # immediately.  strip s reads all 8 T segments.
                    g0 = 2 * pair
                    for s in range(4):
                        j, h = s // 2, s % 2
                        nc.gpsimd.dma_start(
                            out=q64[64 * h:64 * h + 64, g0 + j, 0:M],
                            in_=qsb[32 * s:32 * s + 1, 0:3200])

                units = [l1_quad(0), l1_quad(1), l2_unit(0), l2_unit(1),
                         l1_quad(2), l1_quad(3), l2_unit(2), l2_unit(3),
                         l2_unit(4), l2_unit(5), l1_quad(4), l1_quad(5),
                         l2_unit(6), l2_unit(7), l2_unit(8), l2_unit(9),
                         l1_quad(6), l2_unit(10), l3_quad(0), l3_quad(1),
                         l3_quad(2), l3_quad(3), l2_unit(11), l3_quad(4),
                         l3_quad(5), l2_unit(12), l3_quad(6), l3_quad(7),
                         qflips]
                return units

            def tail_thunks(pair, do_sample):
                """Top-k/stats/sample thunks for this pair's q (already
                flipped into q64 by the previous phase's qflips).

                Returns (pre, late): `pre` is the sort+stats chain emitted at
                the head of the next phase (its deps are ready, so the DVE
                runs it immediately while ACT takes the first drains);
                `late` is sqrt+sample+angle-writeback, emitted a couple of
                units in so the ACT queue head isn't blocked on bn_aggr.
                """
                g0 = 2 * pair
                ths = []

                # top-32 of 50 per (row, group): 4 rounds of max8, with
                # match_replace knocking out the found 8 between rounds.
                # Ping-pong q64[g] <-> srt[g]; q64 is rewritten next step.
                for r in range(4):
                    for g in (g0, g0 + 1):
                        src = (q64, srt, q64, srt)[r]

                        def m8(g=g, r=r, src=src):
                            nc.vector.max(top32[:, 32 * g + 8 * r:
                                                32 * g + 8 * r + 8],
                                          src[:, g, 0:M])
                        ths.append(m8)
                    if r < 3:
                        for g in (g0, g0 + 1):
                            src = (q64, srt, q64)[r]
                            dst = (srt, q64, srt)[r]

                            def mr(g=g, r=r, src=src, dst=dst):
                                nc.vector.match_replace(
                                    dst[:, g, 0:M],
                                    top32[:, 32 * g + 8 * r:
                                          32 * g + 8 * r + 8],
                                    src[:, g, 0:M], NEG)
                            ths.append(mr)

                t32v = top32.rearrange("p (g k) -> p g k", k=NTOP)
                for g in (g0, g0 + 1):
                    def bns(g=g):
                        nc.vector.bn_stats(bnst[:, g, :], t32v[:, g, :])

                    def bna(g=g):
                        nc.vector.bn_aggr(mv[:, g, :], bnst[:, g:g + 1, :])
                    ths += [bns, bna]

                pre, ths = ths, []
                if do_sample:
                    def sqrt_op():
                        nc.scalar.activation(std[:, g0:g0 + 2],
                                             mv[:, g0:g0 + 2, 1], Act.Sqrt,
                                             scale=float(NTOP) / (NTOP - 1))
                    ths.append(sqrt_op)

                    epsv = eps_sbs[pair].rearrange("p (g m) -> p g m", m=M)
                    stdb = std[:, g0:g0 + 2].unsqueeze(2).to_broadcast(
                        (P, 2, M))
                    mub = mv[:, g0:g0 + 2, 0].unsqueeze(2).to_broadcast(
                        (P, 2, M))

                    def smul(epsv=epsv, stdb=stdb):
                        nc.gpsimd.tensor_tensor(tmp_s[:, g0:g0 + 2, :], epsv,
                                                stdb, op=Alu.mult)
                    ths.append(smul)

                    def sadd(mub=mub):
                        nc.gpsimd.tensor_tensor(a_bm[:, g0:g0 + 2, :],
                                                tmp_s[:, g0:g0 + 2, :], mub,
                                                op=Alu.add)
                    ths.append(sadd)

                    def hi_cast():
                        nc.vector.tensor_scalar(a16[:, g0:g0 + 2, 0, :],
                                                a_bm[:, g0:g0 + 2, :],
                                                scalar1=0.0, scalar2=None,
                                                op0=Alu.add)
                    ths.append(hi_cast)

                    def lo_sub():
                        nc.vector.tensor_tensor(a16[:, g0:g0 + 2, 1, :],
                                                a_bm[:, g0:g0 + 2, :],
                                                a16[:, g0:g0 + 2, 0, :],
                                                op=Alu.subtract)
                    ths.append(lo_sub)

                    # per-(group, row) flips: out stream is (row, p, m) so a
                    # merged hi+lo DMA would scramble against a16's (p, row, m)
                    for j in range(2):
                        for r in range(2):
                            def aflip(j=j, r=r):
                                g = g0 + j
                                nc.sync.dma_start(
                                    out=x[2 + r:3 + r,
                                          g * NPG:(g + 1) * NPG].rearrange(
                                        "a (p m) -> a p m", m=M),
                                    in_=a16[:, g, r, :])
                            ths.append(aflip)

                    # replicate the fresh angle rows to partition
                    # stripes 32/64/96 for the row-tiled l1
                    cols = slice(g0 * NPG, (g0 + 2) * NPG)
                    for r in (1, 2, 3):
                        def stripe(r=r, cols=cols):
                            nc.gpsimd.dma_start(
                                out=x[32 * r + 2:32 * r + 4, cols],
                                in_=x[2:4, cols])
                        ths.append(stripe)
                return pre, ths

            def phase(mlp_pair, tail):
                """Emit one pair's MLP with the other pair's tail up front."""
                have_tail = tail is not None
                bal["dve"] = TAIL_DVE_NS if have_tail else 0.0
                bal["act"] = TAIL_ACT_NS if have_tail else 0.0
                pre, late = tail_thunks(*tail) if have_tail else ([], [])
                if mlp_pair is None:
                    for th in pre + late:
                        th()
                    return
                units = mlp_units(mlp_pair)
                for th in pre:
                    th()
                for u in units[:2]:
                    u()
                for th in late:
                    th()
                for u in units[2:]:
                    u()

            # ---- prologue: t = 0 ----
            phase(0, None)
            if n_steps > 1:
                load_eps(0, 0)
            phase(1, (0, n_steps > 1))

            # ---- t = 1 .. : unrolled pipelined loop ----
            n_loop = n_steps - 3
            n_bodies = max(n_loop // BODY_U, 0)
            loop_end = 1 + n_bodies * BODY_U
            if n_bodies > 0:
                with tc.For_i(1, loop_end, BODY_U,
                              hint_engines=(mybir.EngineType.PE,)) as it:
                    for c in range(BODY_U):
                        load_eps(1, it + (c - 1))
                        phase(0, (1, True))
                        load_eps(0, it + c)
                        phase(1, (0, True))

            for t in range(loop_end, n_steps - 1):
                load_eps(1, t - 1)
                phase(0, (1, True))
                load_eps(0, t)
                phase(1, (0, True))

            # ---- epilogue: t = n_steps-1 ----
            if n_steps > 1:
                load_eps(1, n_steps - 2)
                phase(0, (1, True))
                phase(1, (0, False))      # pair A final stats
            phase(None, (1, False))       # pair B final stats
            nc.vector.tensor_scalar(out_sb, mv[:, :, 0], scalar1=TWO_PI,
                                    scalar2=None, op0=Alu.mult)
            nc.sync.dma_start(out=OUT.ap().rearrange("(g p) -> p g", p=P),
                              in_=out_sb)

    nc.compile()
    return nc


def host_prng(n_steps=ITERS - 1):
    """Exactly the reference's PRNG stream, on host CPU."""
    import jax
    import jax.numpy as jnp
    cpu = jax.devices("cpu")[0]
    with jax.default_device(cpu):
        key = jax.device_put(jax.random.key(42), cpu)
        k0, kloop = jax.random.split(key)
        angles0 = np.asarray(jax.random.uniform(k0, (BATCH, M),
                                                dtype=jnp.float32))
        keys = jax.random.split(kloop, ITERS - 1)
        eps = np.stack([
            np.asarray(jax.random.normal(keys[t], (BATCH, M),
                                         dtype=jnp.float32))
            for t in range(max(n_steps - 1, 1))
        ])
    return angles0, eps


def make_in_map(core, states, W1, b1, W2, b2, W3, b3, angles0, eps):
    sl = slice(core * B, (core + 1) * B)
    S = np.ascontiguousarray(states[sl]).reshape(G, P, 2)
    xrep = np.ascontiguousarray(
        np.broadcast_to(S[:, :, None, :], (G, P, M, 2)).transpose(3, 0, 1, 2)
    ).reshape(2, N)
    a0 = np.ascontiguousarray(angles0[sl]).reshape(N).astype(np.float32)
    a0_hi = a0.astype(np.float16)
    a0_lo = (a0 - a0_hi.astype(np.float32)).astype(np.float16)
    stripe = np.concatenate([
        xrep.astype(np.float16),
        a0_hi[None, :],
        a0_lo[None, :],
        np.ones((1, N), np.float16),
    ], axis=0)                      # [5, N]
    w1stripe = np.stack([W1[0], W1[1], W1[2], W1[2], b1]).astype(np.float16)
    xa = np.zeros((101, N), np.float16)
    w1p = np.zeros((101, HIDDEN), np.float16)
    for r in range(4):
        xa[32 * r:32 * r + 5] = stripe
        w1p[32 * r:32 * r + 5] = w1stripe
    nsteps_eps = max(eps.shape[0], 1)
    epsc = np.ascontiguousarray(
        eps[:, sl, :].reshape(nsteps_eps, 2, 2, P, M)
        .transpose(0, 1, 3, 2, 4)
    ).reshape(nsteps_eps, 2, P, 2 * M)
    w2p = np.concatenate([W2, b2[None, :]], axis=0).astype(np.float16)
    w3c = 32
    w3p = np.zeros((HIDDEN + 1, w3c), np.float16)
    w3p[0:HIDDEN, 0] = W3[:, 0].astype(np.float16)
    w3p[HIDDEN, 0] = np.float16(b3[0])
    return {
        "XA": xa,
        "EPS": epsc.astype(np.float32),
        "W1D": w1p,
        "W2D": w2p,
        "W3D": w3p,
    }


LAST_RESULTS = None


def kernel(states, W1, b1, W2, b2, W3, b3, _trace=False):
    global LAST_RESULTS
    from concourse.bass_utils import run_bass_kernel_spmd

    n_steps = ITERS - 1
    if n_steps not in _PROG_CACHE:
        _PROG_CACHE[n_steps] = build_program(n_steps)
    nc = _PROG_CACHE[n_steps]

    angles0, eps = host_prng(n_steps)
    in_maps = [
        make_in_map(c, states, W1, b1, W2, b2, W3, b3, angles0, eps)
        for c in range(NCORES)
    ]
    res = run_bass_kernel_spmd(nc, in_maps, core_ids=list(range(NCORES)),
                               trace=_trace)
    LAST_RESULTS = res
    out = np.concatenate([res.results[c]["OUT"] for c in range(NCORES)])
    return out.astype(np.float32)


# revision 14
# speedup vs baseline: 1.0316x; 1.0046x over previous
"""Trainium2 Bass kernel for nn_DQN CEM sampling problem (v4).

Data-parallel over batch: 4096 rows -> 8 cores x 512 rows. Each core runs the
full 99-step CEM loop on its shard; the tiny MLP weights are replicated.

v4 changes over v3 (trace-driven):
  - The phase's PSUM->SBUF drains (l1+l2 relu, l3 copy; ~33us/phase over
    DVE+ACT) are the hard floor, so the l1/l2/l3 units are emitted
    INTERLEAVED (quad, l2, l2, quad, ...) to keep both drain engines fed
    continuously instead of the v3 see-saw (l1 drain-bound then l2 PE-bound).
  - top-32 via DVE max8 + match_replace (4+3 ops per group) instead of the
    26-op bitonic network: shorter serial tail, ~2x fewer DVE-cycles.
  - sampling arithmetic (eps*std+mu) moved to the otherwise idle GpSimd
    (TT mult/add run on the Q7 cores; max/STT are rejected by walrus).
  - drain balance is tail-aware: DVE's greedy-cost counter starts
    preloaded with the tail work it must also absorb that phase.
  - the tail chain (sort/stats/sample/angle-writeback) is emitted at the
    HEAD of the next phase, not spread through it: its inputs are ready
    (q was flipped by qflips at the end of the owning pair's MLP phase),
    so it runs concurrently with the early drains instead of queueing
    behind all of them and gating the following phase's l1 by ~20us.
  - BODY_U 3 -> 8: each For_i back-edge is an all-engine barrier; since
    angle writeback completes mid-phase, the barrier exposes only ~2us.
"""

import numpy as np

BATCH = 4096
M = 50
NTOP = 32
ITERS = 100  # reference ITERS; device runs ITERS-1 = 99 qnet/stats steps
HIDDEN = 100
NCORES = 8
B = BATCH // NCORES  # 512 rows per core
G = 4                # partition groups per core
P = 128              # rows per group (partitions)
NPG = P * M          # columns per group = 6400
N = G * NPG          # columns per core = 25600
NPAIR = 2 * NPG      # columns per pair = 12800
NEG = -1.0e30
TWO_PI = 6.283185307179586

BODY_U = 8           # CEM steps per For_i body (back-edge = all-engine barrier)

_PROG_CACHE = {}


def _tiles(total, width):
    out = []
    off = 0
    while off < total:
        w = min(width, total - off)
        out.append((off, w))
        off += w
    return out


def build_program(n_steps=ITERS - 1):
    """Build the single-core Bass/Tile program (SPMD across cores)."""
    import concourse.bacc as bacc
    import concourse.bass as bass
    import concourse.tile as tile
    import concourse.mybir as mybir

    f32 = mybir.dt.float32
    fp16 = mybir.dt.float16
    Alu = mybir.AluOpType
    Act = mybir.ActivationFunctionType

    nc = bacc.Bacc("TRN2", target_bir_lowering=False, debug=False)

    XROWS = 101
    XA = nc.dram_tensor("XA", [XROWS, N], fp16, kind="ExternalInput")
    EPS = nc.dram_tensor("EPS", [max(n_steps - 1, 1), 2, P, 2 * M], f32,
                         kind="ExternalInput")
    W1D = nc.dram_tensor("W1D", [XROWS, HIDDEN], fp16, kind="ExternalInput")
    W2D = nc.dram_tensor("W2D", [HIDDEN + 1, HIDDEN], fp16,
                         kind="ExternalInput")
    W3C = 32
    W3D = nc.dram_tensor("W3D", [HIDDEN + 1, W3C], fp16, kind="ExternalInput")
    OUT = nc.dram_tensor("OUT", [B], f32, kind="ExternalOutput")

    sts = _tiles(NPAIR, 1024)      # supertiles for l1/l2 (12x1024 + 512)
    NST = len(sts)                 # 13

    # measured per-op engine costs (ns) for the greedy drain balance
    def dve_cost(w):
        return 175 + 1.20 * w

    def act_cost(w):
        return 180 + 1.13 * w

    # tail work that lands on each engine besides drains (ns), used to
    # preload the balance counters so ACT takes a bigger drain share.
    TAIL_DVE_NS = 3500                           # sort + bn + hi/lo casts
    TAIL_ACT_NS = 300                            # sqrt

    with tile.TileContext(nc) as tc:
        with (
            tc.tile_pool(name="statics", bufs=1) as statics,
            tc.tile_pool(name="hps", bufs=3, space=bass.MemorySpace.PSUM) as hps,
            tc.tile_pool(name="psq", bufs=2, space=bass.MemorySpace.PSUM) as psq,
        ):
            # --- static tiles ---
            # x is SPLIT per pair: the tail of pair Y (emitted at the head of
            # pair X's MLP phase) writes Y's angle rows, and a shared tile
            # would make X's l1 reads order behind those writes.
            x0 = statics.tile([XROWS, NPAIR], fp16)  # s0,s1,a_hi,a_lo,1 (striped)
            x1 = statics.tile([XROWS, NPAIR], fp16)
            xs = (x0, x1)
            h1 = statics.tile([HIDDEN + 1, NPAIR], fp16)
            h2 = statics.tile([HIDDEN + 1, NPAIR], fp16)
            q_sbA = statics.tile([P, NPG], f32)
            q_sbB = statics.tile([P, NPG], f32)
            q64 = statics.tile([P, G, 64], f32)   # batch-major q (50 used)
            srt = statics.tile([P, G, 64], f32)   # match_replace ping-pong
            top32 = statics.tile([P, G * NTOP], f32)
            bnst = statics.tile([P, G, 6], f32)
            mv = statics.tile([P, G, 2], f32)     # (mean, var) per group
            std = statics.tile([P, G], f32)
            a_bm = statics.tile([P, G, M], f32)
            tmp_s = statics.tile([P, G, M], f32)
            a16 = statics.tile([P, G, 2, M], fp16)  # (hi, lo)
            eps_sbA = statics.tile([P, 2 * M], f32)
            eps_sbB = statics.tile([P, 2 * M], f32)
            out_sb = statics.tile([P, G], f32)
            w1s = statics.tile([XROWS, HIDDEN], fp16)
            w2s = statics.tile([HIDDEN + 1, HIDDEN], fp16)
            w3s = statics.tile([HIDDEN + 1, W3C], fp16)

            q_sbs = (q_sbA, q_sbB)
            eps_sbs = (eps_sbA, eps_sbB)

            # --- one-time setup ---
            nc.sync.dma_start(out=w1s, in_=W1D.ap())
            nc.sync.dma_start(out=w2s, in_=W2D.ap())
            nc.sync.dma_start(out=w3s, in_=W3D.ap())
            nc.sync.dma_start(out=x0, in_=XA.ap()[:, 0:NPAIR])
            nc.sync.dma_start(out=x1, in_=XA.ap()[:, NPAIR:N])
            # rows 96..99 are clobbered but rewritten by the first l1/l2
            # drain before any consumer reads them; row 100 stays 1.0
            # (engine APs need a 32-aligned base partition).
            nc.vector.memset(h1[96:HIDDEN + 1, :], 1.0)
            nc.vector.memset(h2[96:HIDDEN + 1, :], 1.0)

            def load_eps(pair, t):
                if isinstance(t, int):
                    src = EPS.ap()[t:t + 1, pair:pair + 1, :, :]
                else:
                    src = EPS.ap()[bass.ds(t, 1), pair:pair + 1, :, :]
                # gpsimd queue: keeps bulky flips off the sync queue's FIFO
                nc.gpsimd.dma_start(out=eps_sbs[pair], in_=src)

            bal = {"dve": 0.0, "act": 0.0}

            def drain(kind, out_ap, in_ap, w):
                """Emit a relu/copy drain on the engine with less queued work."""
                dc = dve_cost(w)
                ac = act_cost(w)
                if bal["dve"] + dc <= bal["act"] + ac:
                    bal["dve"] += dc
                    if kind == "relu":
                        nc.vector.tensor_scalar(out_ap, in_ap, scalar1=0.0,
                                                scalar2=None, op0=Alu.max)
                    else:
                        nc.vector.tensor_copy(out_ap, in_ap)
                else:
                    bal["act"] += ac
                    if kind == "relu":
                        nc.scalar.activation(out_ap, in_ap, Act.Relu)
                    else:
                        nc.scalar.copy(out_ap, in_ap)

            def mlp_units(pair):
                """Interleaved emission units (closures) for this pair's MLP.

                Order: q0 q1 l2_0 q2 l2_1 l2_2 q3 l2_3 l2_4 q4 l2_5 l2_6
                       q5 l2_7 l2_8 q6 l2_9 l2_10 T0 T1 T2 T3 l2_11 T4 T5
                       l2_12 T6 T7 -- keeps PE ~1 supertile ahead of the
                       drains so DVE/ACT (the floor) never starve, and l3
                       tiles start as soon as their h2 columns land.
                """
                xp = xs[pair]
                qsb = q_sbs[pair]

                def l1_quad(qi):
                    # 4x row-tiled: quad of concurrent matmuls covers two
                    # 1024-supertiles; stripe r reads x/W1 at partitions 32r.
                    pair_sts = sts[2 * qi:2 * qi + 2]

                    def emit():
                        tiles_ = []
                        for _ in pair_sts:
                            hst = hps.tile([HIDDEN, 1024], f32,
                                           tag="hst", name="hst")
                            tiles_.append(hst)
                        r = 0
                        for ti, (off, w) in enumerate(pair_sts):
                            for w0 in range(0, w, 512):
                                ww = min(512, w - w0)
                                c0 = off + w0
                                nc.tensor.matmul(
                                    tiles_[ti][:, w0:w0 + ww],
                                    w1s[32 * r:32 * r + 5, :],
                                    xp[32 * r:32 * r + 5, c0:c0 + ww],
                                    tile_position=(32 * r, 0))
                                r += 1
                        for ti, (off, w) in enumerate(pair_sts):
                            drain("relu", h1[0:HIDDEN, off:off + w],
                                  tiles_[ti][:, 0:w], w)
                    return emit

                def l2_unit(k):
                    off, w = sts[k]

                    def emit():
                        st = hps.tile([HIDDEN, 1024], f32, tag="hst")
                        for w0 in range(0, w, 512):
                            ww = min(512, w - w0)
                            nc.tensor.matmul(st[:, w0:w0 + ww], w2s,
                                             h1[:, off + w0:off + w0 + ww])
                        drain("relu", h2[0:HIDDEN, off:off + w], st[:, 0:w], w)
                    return emit

                def l3_quad(T):
                    # 4x col-tiled: strip s = (group j, row-half h); tile
                    # tau = 8h+T covers rows [8*tau, 8*tau+8) of group j, so
                    # each strip's q is a contiguous 64-partition run after
                    # the flip (nested partition APs don't lower for DMA).
                    def emit():
                        qp = psq.tile([P, 400], f32, tag="qp")
                        for s in range(4):
                            j, h = s // 2, s % 2
                            tau = 8 * h + T
                            c0 = j * NPG + tau * 400
                            nc.tensor.matmul(
                                qp[32 * s:32 * s + 32, :], w3s,
                                h2[:, c0:c0 + 400],
                                tile_position=(0, 32 * s))
                        drain("copy", qsb[:, 400 * T:400 * T + 400],
                              qp[:, :], 400)
                    return emit

                def qflips():
                    # flip q to batch-major as soon as the last l3 copy
                    # lands, so the next phase's tail starts with max8
                    # immediately.  strip s reads all 8 T segments.
                    g0 = 2 * pair
                    for s in range(4):
                        j, h = s // 2, s % 2
                        nc.gpsimd.dma_start(
                            out=q64[64 * h:64 * h + 64, g0 + j, 0:M],
                            in_=qsb[32 * s:32 * s + 1, 0:3200])

                units = [l1_quad(0), l1_quad(1), l2_unit(0), l2_unit(1),
                         l1_quad(2), l1_quad(3), l2_unit(2), l2_unit(3),
                         l2_unit(4), l2_unit(5), l1_quad(4), l1_quad(5),
                         l2_unit(6), l2_unit(7), l2_unit(8), l2_unit(9),
                         l1_quad(6), l2_unit(10), l3_quad(0), l3_quad(1),
                         l3_quad(2), l3_quad(3), l2_unit(11), l3_quad(4),
                         l3_quad(5), l2_unit(12), l3_quad(6), l3_quad(7),
                         qflips]
                return units

            def tail_thunks(pair, do_sample):
                """Top-k/stats/sample thunks for this pair's q (already
                flipped into q64 by the previous phase's qflips).

                Returns (pre, late): `pre` is the sort+stats chain emitted at
                the head of the next phase (its deps are ready, so the DVE
                runs it immediately while ACT takes the first drains);
                `late` is sqrt+sample+angle-writeback, emitted a couple of
                units in so the ACT queue head isn't blocked on bn_aggr.
                """
                g0 = 2 * pair
                ths = []

                # top-32 of 50 per (row, group): 4 rounds of max8, with
                # match_replace knocking out the found 8 between rounds.
                # Ping-pong q64[g] <-> srt[g]; q64 is rewritten next step.
                for r in range(4):
                    for g in (g0, g0 + 1):
                        src = (q64, srt, q64, srt)[r]

                        def m8(g=g, r=r, src=src):
                            nc.vector.max(top32[:, 32 * g + 8 * r:
                                                32 * g + 8 * r + 8],
                                          src[:, g, 0:M])
                        ths.append(m8)
                    if r < 3:
                        for g in (g0, g0 + 1):
                            src = (q64, srt, q64)[r]
                            dst = (srt, q64, srt)[r]

                            def mr(g=g, r=r, src=src, dst=dst):
                                nc.vector.match_replace(
                                    dst[:, g, 0:M],
                                    top32[:, 32 * g + 8 * r:
                                          32 * g + 8 * r + 8],
                                    src[:, g, 0:M], NEG)
                            ths.append(mr)

                t32v = top32.rearrange("p (g k) -> p g k", k=NTOP)
                for g in (g0, g0 + 1):
                    def bns(g=g):
                        nc.vector.bn_stats(bnst[:, g, :], t32v[:, g, :])

                    def bna(g=g):
                        nc.vector.bn_aggr(mv[:, g, :], bnst[:, g:g + 1, :])
                    ths += [bns, bna]

                pre, ths = ths, []
                if do_sample:
                    def sqrt_op():
                        nc.scalar.activation(std[:, g0:g0 + 2],
                                             mv[:, g0:g0 + 2, 1], Act.Sqrt,
                                             scale=float(NTOP) / (NTOP - 1))
                    ths.append(sqrt_op)

                    epsv = eps_sbs[pair].rearrange("p (g m) -> p g m", m=M)
                    stdb = std[:, g0:g0 + 2].unsqueeze(2).to_broadcast(
                        (P, 2, M))
                    mub = mv[:, g0:g0 + 2, 0].unsqueeze(2).to_broadcast(
                        (P, 2, M))

                    def smul(epsv=epsv, stdb=stdb):
                        nc.gpsimd.tensor_tensor(tmp_s[:, g0:g0 + 2, :], epsv,
                                                stdb, op=Alu.mult)
                    ths.append(smul)

                    def sadd(mub=mub):
                        nc.gpsimd.tensor_tensor(a_bm[:, g0:g0 + 2, :],
                                                tmp_s[:, g0:g0 + 2, :], mub,
                                                op=Alu.add)
                    ths.append(sadd)

                    def hi_cast():
                        nc.vector.tensor_scalar(a16[:, g0:g0 + 2, 0, :],
                                                a_bm[:, g0:g0 + 2, :],
                                                scalar1=0.0, scalar2=None,
                                                op0=Alu.add)
                    ths.append(hi_cast)

                    def lo_sub():
                        nc.vector.tensor_tensor(a16[:, g0:g0 + 2, 1, :],
                                                a_bm[:, g0:g0 + 2, :],
                                                a16[:, g0:g0 + 2, 0, :],
                                                op=Alu.subtract)
                    ths.append(lo_sub)

                    # per-(group, row) flips: out stream is (row, p, m) so a
                    # merged hi+lo DMA would scramble against a16's (p, row, m)
                    xp = xs[pair]
                    for j in range(2):
                        for r in range(2):
                            def aflip(j=j, r=r):
                                nc.sync.dma_start(
                                    out=xp[2 + r:3 + r,
                                           j * NPG:(j + 1) * NPG].rearrange(
                                        "a (p m) -> a p m", m=M),
                                    in_=a16[:, g0 + j, r, :])
                            ths.append(aflip)

                    # replicate the fresh angle rows to partition
                    # stripes 32/64/96 for the row-tiled l1
                    for r in (1, 2, 3):
                        def stripe(r=r):
                            nc.gpsimd.dma_start(
                                out=xp[32 * r + 2:32 * r + 4, :],
                                in_=xp[2:4, :])
                        ths.append(stripe)
                return pre, ths

            def phase(mlp_pair, tail):
                """Emit one pair's MLP with the other pair's tail up front."""
                have_tail = tail is not None
                bal["dve"] = TAIL_DVE_NS if have_tail else 0.0
                bal["act"] = TAIL_ACT_NS if have_tail else 0.0
                pre, late = tail_thunks(*tail) if have_tail else ([], [])
                if mlp_pair is None:
                    for th in pre + late:
                        th()
                    return
                units = mlp_units(mlp_pair)
                for th in pre:
                    th()
                for u in units[:2]:
                    u()
                for th in late:
                    th()
                for u in units[2:]:
                    u()

            # ---- prologue: t = 0 ----
            phase(0, None)
            if n_steps > 1:
                load_eps(0, 0)
            phase(1, (0, n_steps > 1))

            # ---- t = 1 .. : unrolled pipelined loop ----
            n_loop = n_steps - 3
            n_bodies = max(n_loop // BODY_U, 0)
            loop_end = 1 + n_bodies * BODY_U
            if n_bodies > 0:
                with tc.For_i(1, loop_end, BODY_U,
                              hint_engines=(mybir.EngineType.PE,)) as it:
                    for c in range(BODY_U):
                        load_eps(1, it + (c - 1))
                        phase(0, (1, True))
                        load_eps(0, it + c)
                        phase(1, (0, True))

            for t in range(loop_end, n_steps - 1):
                load_eps(1, t - 1)
                phase(0, (1, True))
                load_eps(0, t)
                phase(1, (0, True))

            # ---- epilogue: t = n_steps-1 ----
            if n_steps > 1:
                load_eps(1, n_steps - 2)
                phase(0, (1, True))
                phase(1, (0, False))      # pair A final stats
            phase(None, (1, False))       # pair B final stats
            nc.vector.tensor_scalar(out_sb, mv[:, :, 0], scalar1=TWO_PI,
                                    scalar2=None, op0=Alu.mult)
            nc.sync.dma_start(out=OUT.ap().rearrange("(g p) -> p g", p=P),
                              in_=out_sb)

    nc.compile()
    return nc


def host_prng(n_steps=ITERS - 1):
    """Exactly the reference's PRNG stream, on host CPU."""
    import jax
    import jax.numpy as jnp
    cpu = jax.devices("cpu")[0]
    with jax.default_device(cpu):
        key = jax.device_put(jax.random.key(42), cpu)
        k0, kloop = jax.random.split(key)
        angles0 = np.asarray(jax.random.uniform(k0, (BATCH, M),
                                                dtype=jnp.float32))
        keys = jax.random.split(kloop, ITERS - 1)
        eps = np.stack([
            np.asarray(jax.random.normal(keys[t], (BATCH, M),
                                         dtype=jnp.float32))
            for t in range(max(n_steps - 1, 1))
        ])
    return angles0, eps


def make_in_map(core, states, W1, b1, W2, b2, W3, b3, angles0, eps):
    sl = slice(core * B, (core + 1) * B)
    S = np.ascontiguousarray(states[sl]).reshape(G, P, 2)
    xrep = np.ascontiguousarray(
        np.broadcast_to(S[:, :, None, :], (G, P, M, 2)).transpose(3, 0, 1, 2)
    ).reshape(2, N)
    a0 = np.ascontiguousarray(angles0[sl]).reshape(N).astype(np.float32)
    a0_hi = a0.astype(np.float16)
    a0_lo = (a0 - a0_hi.astype(np.float32)).astype(np.float16)
    stripe = np.concatenate([
        xrep.astype(np.float16),
        a0_hi[None, :],
        a0_lo[None, :],
        np.ones((1, N), np.float16),
    ], axis=0)                      # [5, N]
    w1stripe = np.stack([W1[0], W1[1], W1[2], W1[2], b1]).astype(np.float16)
    xa = np.zeros((101, N), np.float16)
    w1p = np.zeros((101, HIDDEN), np.float16)
    for r in range(4):
        xa[32 * r:32 * r + 5] = stripe
        w1p[32 * r:32 * r + 5] = w1stripe
    nsteps_eps = max(eps.shape[0], 1)
    epsc = np.ascontiguousarray(
        eps[:, sl, :].reshape(nsteps_eps, 2, 2, P, M)
        .transpose(0, 1, 3, 2, 4)
    ).reshape(nsteps_eps, 2, P, 2 * M)
    w2p = np.concatenate([W2, b2[None, :]], axis=0).astype(np.float16)
    w3c = 32
    w3p = np.zeros((HIDDEN + 1, w3c), np.float16)
    w3p[0:HIDDEN, 0] = W3[:, 0].astype(np.float16)
    w3p[HIDDEN, 0] = np.float16(b3[0])
    return {
        "XA": xa,
        "EPS": epsc.astype(np.float32),
        "W1D": w1p,
        "W2D": w2p,
        "W3D": w3p,
    }


LAST_RESULTS = None


def kernel(states, W1, b1, W2, b2, W3, b3, _trace=False):
    global LAST_RESULTS
    from concourse.bass_utils import run_bass_kernel_spmd

    n_steps = ITERS - 1
    if n_steps not in _PROG_CACHE:
        _PROG_CACHE[n_steps] = build_program(n_steps)
    nc = _PROG_CACHE[n_steps]

    angles0, eps = host_prng(n_steps)
    in_maps = [
        make_in_map(c, states, W1, b1, W2, b2, W3, b3, angles0, eps)
        for c in range(NCORES)
    ]
    res = run_bass_kernel_spmd(nc, in_maps, core_ids=list(range(NCORES)),
                               trace=_trace)
    LAST_RESULTS = res
    out = np.concatenate([res.results[c]["OUT"] for c in range(NCORES)])
    return out.astype(np.float32)
